# revision 5
# baseline (speedup 1.0000x reference)
"""Causal self-attention Trainium2 Bass kernel.

Problem: B=4, S=2048, C=1024, H=16 heads, D=64 head_dim.
  qkv = x @ qkv_w.T + qkv_b ; per-head causal softmax attention ; out = attn @ proj_w.T + proj_b

Sharding (8 cores): core = 2*b + hg  (data parallel over batch b=0..3,
tensor parallel over 2 head-groups of 8 heads).  Each core computes
q/k/v for its 8 heads over the full sequence, does causal attention
locally, and computes a partial output projection (contraction over its
512 channels).  Host sums the two partials per batch.

Device layout choices (all matmuls bf16 inputs, fp32 PSUM accumulate):
  - qkvT produced in transposed [c', s] orientation directly from the
    projection (lhsT=wT tile, rhs=xT tile), so per-head qT/kT tiles
    [d=64, s] are ready for the scores matmul with zero transposes.
  - scores computed transposed: sT[k,q] = kT.T @ qT (contraction d on
    partitions).  Softmax sums over k (partition dim) come for free from
    a ones-column appended to v in the AV matmul.  No max-subtraction
    (init scale 0.02 keeps |scores| < ~4, exp is safe).
  - v produced in [s, c'] orientation (lhsT=xT tile, rhs=wT tile) which
    is exactly the AV lhsT layout.
  - causal masking: full 128x512 score blocks beyond the diagonal are
    skipped; the 4 diagonal blocks per q-chunk are multiplied by a
    precomputed 0/1 mask after exp.
"""

import numpy as np
import ml_dtypes

import concourse.bacc as bacc
import concourse.bass as bass
import concourse.mybir as mybir
import concourse.tile as tile
from concourse.bass_utils import run_bass_kernel_spmd

BF = ml_dtypes.bfloat16
F32 = mybir.dt.float32
BF16 = mybir.dt.bfloat16
EXP = mybir.ActivationFunctionType.Exp

B, S, C = 4, 2048, 1024
H, D = 16, 64
P = 128
NQ = 512            # q-chunk (psum bank free size)
NSQ = S // NQ       # 4 q-chunks
NKB = S // P        # 16 k-blocks
CO = C // P         # 8 contraction tiles for stage 1
CPH = 512           # channels per head-group (8 heads * 64)

LAST_RESULTS = None
_NC_CACHE = []


def _ensure_axon_hooks():
    """Provide antenv.axon_hooks (NTFF profile hook) when the image lacks it.

    concourse.bass_utils imports it unconditionally on the trace path; this
    container's antenv has no axon_hooks module, but the axon PJRT .so does
    export the profiling C ABI.  Recreates the slim ctypes hook from
    trn_boot._ntff_profile_via_ctypes.  Also stubs out the S3 artifact
    upload (no credentials in-container).
    """
    import sys
    import types
    import contextlib
    import ctypes
    import os

    from concourse import bass_utils as _bu
    _bu.upload_artifacts = lambda tmpdir: str(tmpdir)

    try:
        import antenv.axon_hooks  # noqa: F401
        return
    except ImportError:
        pass

    state = {}

    def set_axon_ntff_profile_hook(hook):
        state["hook"] = hook

    def get_axon_ntff_profile_hook():
        if "hook" in state:
            return state["hook"]
        so = "/opt/axon/libaxon_pjrt.so"
        if not os.path.exists(so):
            return None
        lib = ctypes.CDLL(so)
        if not hasattr(lib, "axon_start_nrt_profile"):
            return None
        lib.axon_start_nrt_profile.argtypes = [
            ctypes.POINTER(ctypes.c_int64), ctypes.c_size_t]
        lib.axon_start_nrt_profile.restype = ctypes.c_int64
        lib.axon_stop_nrt_profile.argtypes = [ctypes.c_char_p]
        lib.axon_stop_nrt_profile.restype = ctypes.c_int64

        @contextlib.contextmanager
        def _hook(output_dir, device_ids):
            import jax
            jax.devices()
            if device_ids:
                ids = (ctypes.c_int64 * len(device_ids))(*device_ids)
                rc = lib.axon_start_nrt_profile(ids, len(device_ids))
            else:
                rc = lib.axon_start_nrt_profile(None, 0)
            if rc != 0:
                raise RuntimeError(f"axon_start_nrt_profile rc={rc}")
            try:
                yield
            finally:
                n = lib.axon_stop_nrt_profile(str(output_dir).encode())
                print(f"ntff profile: {n} file(s) written to {output_dir}")

        state["hook"] = _hook
        return _hook

    import antenv
    mod = types.ModuleType("antenv.axon_hooks")
    mod.set_axon_ntff_profile_hook = set_axon_ntff_profile_hook
    mod.get_axon_ntff_profile_hook = get_axon_ntff_profile_hook
    sys.modules["antenv.axon_hooks"] = mod
    antenv.axon_hooks = mod


def _build_program():
    nc = bacc.Bacc("TRN2", target_bir_lowering=False, debug=False)

    xT = nc.dram_tensor("xT", [C, S], BF16, kind="ExternalInput")            # [c, s]
    wT = nc.dram_tensor("wT", [C, 3 * CPH], BF16, kind="ExternalInput")      # [c, c'] cols: q|k|v
    qkb = nc.dram_tensor("qkb", [P, 8], F32, kind="ExternalInput")           # q,k bias, partition-major
    bvb = nc.dram_tensor("bvb", [P, CPH], F32, kind="ExternalInput")         # v bias bcast over partitions
    pwT = nc.dram_tensor("pwT", [CPH, C], BF16, kind="ExternalInput")        # [ci, co]
    pbb = nc.dram_tensor("pbb", [P, C], F32, kind="ExternalInput")           # proj bias bcast (zeros on hg=1)
    dmask = nc.dram_tensor("dmask", [P, 4, NQ], BF16, kind="ExternalInput")  # causal 0/1 diag-block mask
    out = nc.dram_tensor("out", [S, C], F32, kind="ExternalOutput")

    xT_r = xT.rearrange("(o p) s -> p o s", p=P)
    wT_r = wT.rearrange("(o p) m -> p o m", p=P)
    pwT_r = pwT.rearrange("(o p) m -> p o m", p=P)

    with tile.TileContext(nc) as tc:
        with (
            tc.tile_pool(name="const", bufs=1) as const,
            tc.tile_pool(name="work", bufs=4) as work,
            tc.tile_pool(name="psg", bufs=2, space="PSUM") as psum_gen,
            tc.tile_pool(name="pss", bufs=2, space="PSUM") as psum_sc,
            tc.tile_pool(name="psa", bufs=2, space="PSUM") as psum_av,
        ):
            # ---- persistent SBUF + input DMAs ----
            xT_sb = const.tile([P, CO, S], BF16, tag="xT", name="xT_sb")
            wT_sb = const.tile([P, CO, 3 * CPH], BF16, tag="wT", name="wT_sb")
            for o in range(CO):
                nc.sync.dma_start(out=wT_sb[:, o, :], in_=wT_r[:, o, :])
                nc.sync.dma_start(out=xT_sb[:, o, :], in_=xT_r[:, o, :])
            qkb_sb = const.tile([P, 8], F32, tag="qkb", name="qkb_sb")
            nc.sync.dma_start(out=qkb_sb, in_=qkb[:, :])
            bvb_sb = const.tile([P, CPH], F32, tag="bvb", name="bvb_sb")
            nc.sync.dma_start(out=bvb_sb, in_=bvb[:, :])
            pwT_sb = const.tile([P, CPH // P, C], BF16, tag="pwT", name="pwT_sb")
            for o in range(CPH // P):
                nc.sync.dma_start(out=pwT_sb[:, o, :], in_=pwT_r[:, o, :])
            pbb_sb = const.tile([P, C], F32, tag="pbb", name="pbb_sb")
            nc.sync.dma_start(out=pbb_sb, in_=pbb[:, :])
            dm_sb = const.tile([P, 4, NQ], BF16, tag="dmask", name="dm_sb")
            nc.sync.dma_start(out=dm_sb, in_=dmask[:, :, :])

            ones_sb = const.tile([1, D], F32, tag="ones", name="ones_sb")
            nc.vector.memset(ones_sb, 1.0)

            # per-head-pair persistent tensors
            qT_sb = [const.tile([P, S], BF16, tag=f"qT{p}", name=f"qT_sb{p}") for p in range(4)]
            kT_sb = [const.tile([P, S], BF16, tag=f"kT{p}", name=f"kT_sb{p}") for p in range(4)]
            # v: [s-part, kb, parity, d+ones]
            v_sb = [const.tile([P, NKB, 2, D + 1], BF16, tag=f"v{p}", name=f"v_sb{p}") for p in range(4)]
            aT_sb = [const.tile([P, S], BF16, tag=f"aT{p}", name=f"aT_sb{p}") for p in range(4)]
            for p in range(4):
                nc.vector.memset(v_sb[p][:, :, :, D:D + 1], 1.0)

            def stage1_qk(co):
                """c'-tile co of qkvT (co 0..3 -> qT pair, 4..7 -> kT pair)."""
                dst = qT_sb[co] if co < 4 else kT_sb[co - 4]
                for sq in range(NSQ):
                    ps = psum_gen.tile([P, NQ], F32, tag="gen", name=f"ps_qk_{co}_{sq}")
                    for kc in range(CO):
                        nc.tensor.matmul(
                            ps,
                            lhsT=wT_sb[:, kc, co * P:(co + 1) * P],
                            rhs=xT_sb[:, kc, sq * NQ:(sq + 1) * NQ],
                            start=(kc == 0), stop=(kc == CO - 1),
                        )
                    nc.vector.tensor_scalar_add(
                        out=dst[:, sq * NQ:(sq + 1) * NQ], in0=ps,
                        scalar1=qkb_sb[:, co:co + 1],
                    )

            def stage1_v():
                for st in range(NKB):
                    ps = psum_gen.tile([P, CPH], F32, tag="gen", name=f"ps_v_{st}")
                    for kc in range(CO):
                        nc.tensor.matmul(
                            ps,
                            lhsT=xT_sb[:, kc, st * P:(st + 1) * P],
                            rhs=wT_sb[:, kc, 2 * CPH:3 * CPH],
                            start=(kc == 0), stop=(kc == CO - 1),
                        )
                    for p in range(4):
                        nc.vector.tensor_add(
                            out=v_sb[p][:, st, :, 0:D],
                            in0=ps[:, p * P:(p + 1) * P].rearrange("q (a b) -> q a b", a=2),
                            in1=bvb_sb[:, p * P:(p + 1) * P].rearrange("q (a b) -> q a b", a=2),
                        )

            def attention_pair(pr):
                for q0 in range(NSQ):
                    qs = slice(q0 * NQ, (q0 + 1) * NQ)
                    avs = [
                        psum_av.tile([D + 1, NQ], F32, tag="av", name=f"av_{pr}_{q0}_{par}")
                        for par in range(2)
                    ]
                    ngrp = 2 * (q0 + 1)          # groups of 2 k-blocks
                    for g in range(ngrp):
                        pts = []
                        for par in range(2):
                            base = par * D
                            ps = psum_sc.tile([P, 2, NQ], F32, tag="sc",
                                              name=f"ps_sc_{pr}_{q0}_{g}_{par}")
                            for i in range(2):
                                kb = 2 * g + i
                                nc.tensor.matmul(
                                    ps[:, i, :],
                                    lhsT=kT_sb[pr][base:base + D, kb * P:(kb + 1) * P],
                                    rhs=qT_sb[pr][base:base + D, qs],
                                    start=True, stop=True,
                                )
                            pt = work.tile([P, 2, NQ], BF16, tag="pt",
                                           name=f"pt_{pr}_{q0}_{g}_{par}")
                            nc.scalar.activation(out=pt, in_=ps, func=EXP, scale=0.125)
                            pts.append(pt)
                        if g >= ngrp - 2:        # diagonal groups need causal mask
                            r0 = (g - (ngrp - 2)) * 2
                            for pt in pts:
                                nc.vector.tensor_mul(out=pt, in0=pt, in1=dm_sb[:, r0:r0 + 2, :])
                        for par in range(2):
                            for i in range(2):
                                kb = 2 * g + i
                                nc.tensor.matmul(
                                    avs[par],
                                    lhsT=v_sb[pr][:, kb, par, :],
                                    rhs=pts[par][:, i, :],
                                    start=(g == 0 and i == 0),
                                    stop=(g == ngrp - 1 and i == 1),
                                )
                    for par in range(2):
                        av = avs[par]
                        rec = work.tile([1, NQ], F32, tag="rec", name=f"rec_{pr}_{q0}_{par}")
                        nc.vector.reciprocal(out=rec, in_=av[D:D + 1, :])
                        bc = psum_gen.tile([D, NQ], F32, tag="gen", name=f"bc_{pr}_{q0}_{par}")
                        nc.tensor.matmul(bc, lhsT=ones_sb, rhs=rec, start=True, stop=True)
                        bcs = work.tile([D, NQ], F32, tag="bcs", name=f"bcs_{pr}_{q0}_{par}")
                        nc.vector.tensor_copy(out=bcs, in_=bc)
                        nc.vector.tensor_mul(
                            out=aT_sb[pr][par * D:(par + 1) * D, qs],
                            in0=av[0:D, :], in1=bcs,
                        )

            def stage3():
                for st in range(NKB):
                    for c2 in range(2):
                        ps = psum_gen.tile([P, NQ], F32, tag="gen", name=f"ps_o_{st}_{c2}")
                        for o in range(4):
                            nc.tensor.matmul(
                                ps,
                                lhsT=aT_sb[o][:, st * P:(st + 1) * P],
                                rhs=pwT_sb[:, o, c2 * NQ:(c2 + 1) * NQ],
                                start=(o == 0), stop=(o == 3),
                            )
                        ot = work.tile([P, NQ], F32, tag="out", name=f"ot_{st}_{c2}")
                        nc.vector.tensor_add(out=ot, in0=ps, in1=pbb_sb[:, c2 * NQ:(c2 + 1) * NQ])
                        nc.sync.dma_start(
                            out=out[st * P:(st + 1) * P, c2 * NQ:(c2 + 1) * NQ], in_=ot,
                        )

            # emission order chosen so ACT (softmax exp) starts early and
            # stage-1/3 PE work fills ACT-bound stretches of attention
            for pr in range(4):
                stage1_qk(pr)        # qT pair pr
                stage1_qk(4 + pr)    # kT pair pr
                if pr == 0:
                    stage1_v()
                attention_pair(pr)
            stage3()

    nc.compile()
    return nc


def _get_nc():
    if not _NC_CACHE:
        _NC_CACHE.append(_build_program())
    return _NC_CACHE[0]


def _make_in_maps(x, qkv_w, qkv_b, proj_w, proj_b):
    x = np.asarray(x, np.float32)
    qkv_w = np.asarray(qkv_w, np.float32)
    qkv_b = np.asarray(qkv_b, np.float32)
    proj_w = np.asarray(proj_w, np.float32)
    proj_b = np.asarray(proj_b, np.float32)

    # causal mask for the 4 diagonal 128x512 blocks of a q-chunk (k <= q)
    kk = np.arange(4)[None, :, None] * P + np.arange(P)[:, None, None]
    qq = np.arange(NQ)[None, None, :]
    dmask = (kk <= qq).astype(BF)

    in_maps = []
    for core in range(8):
        b, hg = core // 2, core % 2
        rows = slice(hg * CPH, (hg + 1) * CPH)
        w_shard = np.concatenate(
            [qkv_w[0 * C:][rows], qkv_w[1 * C:][rows], qkv_w[2 * C:][rows]], axis=0
        )  # [1536, 1024]
        bq = qkv_b[0 * C:][rows]
        bk = qkv_b[1 * C:][rows]
        bv = qkv_b[2 * C:][rows]
        in_maps.append({
            "xT": np.ascontiguousarray(x[b].T).astype(BF),
            "wT": np.ascontiguousarray(w_shard.T).astype(BF),
            "qkb": np.ascontiguousarray(
                np.concatenate([bq, bk]).reshape(8, P).T).astype(np.float32),
            "bvb": np.ascontiguousarray(np.tile(bv[None, :], (P, 1))).astype(np.float32),
            "pwT": np.ascontiguousarray(proj_w[:, rows].T).astype(BF),
            "pbb": (np.tile(proj_b[None, :], (P, 1)).astype(np.float32)
                    if hg == 0 else np.zeros((P, C), np.float32)),
            "dmask": dmask,
        })
    return in_maps


def kernel(x, qkv_w, qkv_b, proj_w, proj_b, _trace=False):
    global LAST_RESULTS
    _ensure_axon_hooks()
    in_maps = _make_in_maps(x, qkv_w, qkv_b, proj_w, proj_b)
    nc = _get_nc()
    res = run_bass_kernel_spmd(nc, in_maps, core_ids=list(range(8)), trace=_trace)
    LAST_RESULTS = res
    out = np.empty((B, S, C), np.float32)
    for b in range(B):
        out[b] = res.results[2 * b]["out"] + res.results[2 * b + 1]["out"]
    return out
